# revision 1
# baseline (speedup 1.0000x reference)
"""MiniBatchDiscrimination Trainium2 kernel (8-core SPMD).

Reference computation:
    m = (x @ T).reshape(B, OUT_F, NUM_K)            # B=256, OUT_F=128, NUM_K=16
    dists = |m[None,:,:,:] - m[:,None,:,:]|         # [B, B, OUT_F, NUM_K]
    out = sum_i exp(-sum_k dists) - 1               # [B, OUT_F]
    return concat([x, out], axis=-1)                # [B, 640]

Strategy (per core, identical SPMD program, per-core data):
  * Each core owns JB=32 output rows (j). Full m is computed on every core
    (replicated GEMM, cheap) so no collectives are needed.
  * m is stored in SBUF as [partition p=(f8,k), free n=(i,f_o)] with
    f = f_o*8 + f8, p = f8*16 + k. Then T's columns c = f*16+k satisfy
    c = f_o*128 + p, i.e. each f_o corresponds to a contiguous 128-column
    block of T -> the GEMM producing this layout is 16 plain matmuls.
  * Pairwise pass per i: DVE bf16 tensor_sub (2x mode) of m_shard vs
    m_all[:,i] broadcast over j; abs on ACT (Abs activation) or DVE
    (tensor_scalar abs_max, 4x mode) -- split tunable; then the k-sum is a
    TensorE matmul with a block-diagonal ones [128,8] writing dist rows for
    16 consecutive i into one PSUM bank [128=(i_sub,f8), 512=(j,f_o)].
  * exp(-dist): ACT Exp with scale=-1 from PSUM -> bf16 SBUF.
  * sum over i: TensorE matmuls with a second ones pattern accumulating in
    PSUM across all 16 blocks -> [8, 512] = out[f8, (j, f_o)].
  * Host unshards: reshape to [32,128] per core, concat with x.
"""

import os
import numpy as np

import concourse.bass as bass
import concourse.tile as tile
from concourse import bacc, mybir

BF16 = mybir.dt.bfloat16
FP32 = mybir.dt.float32
NPBF16 = np.dtype(mybir.dt.np(BF16))

B = 256
IN_F = 512
OUT_F = 128
NUM_K = 16
N_CORES = 8
JB = B // N_CORES          # 32 j-rows owned per core
F8 = 8                     # f8 = f % 8   (partition group)
FO = OUT_F // F8           # 16 f_o values (free dim)
KC = IN_F // 128           # 4 contraction chunks for the GEMM
NBLK = B // 16             # 16 i-blocks of 16
SUBI = 8                   # i's per DVE sub instruction
# per 8-i sub-batch, how many i's of the abs go to the scalar engine
# (the rest run on DVE as 4x-mode sign-bit stripping)
ACT_ABS_N = int(os.environ.get("ACT_ABS_N", "5"))


def build_nc():
    nc = bacc.Bacc(name="minibatch_discrim")

    # host-prearranged [p, c, i] so each partition's DMA line is contiguous;
    # columns B..B+JB repeat this core's own j-columns so one FD=288 matmul
    # produces m_all and m_sh together (identical values -> exact diagonal).
    xT_d = nc.dram_tensor("xT", [128, KC, B + JB], BF16, kind="ExternalInput")
    # T pre-permuted on host to [fo][p][c][128 cols] so each fo block is
    # one contiguous 128KB DMA (1KB per partition line) that unblocks that
    # fo's GEMM immediately.
    T_d = nc.dram_tensor("T_w", [FO, 128, KC, 128], BF16, kind="ExternalInput")
    onk_d = nc.dram_tensor("ones_k", [128, 8 * 64], BF16, kind="ExternalInput")
    ona_d = nc.dram_tensor("ones_acc", [128, F8], BF16, kind="ExternalInput")
    out_d = nc.dram_tensor("out_pair", [F8, JB * FO], FP32, kind="ExternalOutput")

    with tile.TileContext(nc) as tc:
        with (
            tc.tile_pool(name="const", bufs=1) as constp,
            tc.tile_pool(name="mm", bufs=1) as mmp,
            tc.tile_pool(name="gpsum", bufs=4, space=bass.MemorySpace.PSUM) as gps,
            tc.tile_pool(name="dpsum", bufs=3, space=bass.MemorySpace.PSUM) as dps,
            tc.tile_pool(name="apsum", bufs=1, space=bass.MemorySpace.PSUM) as aps,
            tc.tile_pool(name="work", bufs=3) as wp,
            tc.tile_pool(name="expp", bufs=3) as ep,
        ):
            # ---- constants / inputs to SBUF ----
            zero_b = constp.tile([128, 1], FP32)
            nc.gpsimd.memset(zero_b[:], 0.0)
            neg1_b = constp.tile([128, 1], FP32)
            nc.gpsimd.memset(neg1_b[:], -1.0)

            # ones_k[:, q8, (q, f8)] = 1 iff q == q8 and p//16 == f8.
            # The k-reduce matmul for i_sub targets the 64-partition slice at
            # offset (isub//8)*64 using pattern q8 = isub%8: its 8 target rows
            # get sum_k, the other 56 rows of the slice accumulate += 0.
            ones_k = constp.tile([128, 8, 64], BF16)
            nc.sync.dma_start(ones_k[:], onk_d.rearrange("p (s q) -> p s q", q=64))
            ones_a = constp.tile([128, F8], BF16)
            nc.sync.dma_start(ones_a[:], ona_d[:])

            # warm the ACT exp/abs table while DMAs run
            warm = constp.tile([128, 1], FP32)
            nc.scalar.activation(
                warm[:], zero_b[:], mybir.ActivationFunctionType.Exp, bias=zero_b[:]
            )

            # xT as [p, c, i]  (contraction chunk c)
            xT_sb = constp.tile([128, KC, B + JB], BF16)
            nc.sync.dma_start(xT_sb[:], xT_d[:])
            # T per-fo tiles; 16 independent DMAs so fo-GEMMs start as soon
            # as their slice lands.
            T_tiles = []
            for fo in range(FO):
                tt = constp.tile([128, KC, 128], BF16, tag=f"T{fo}")
                nc.sync.dma_start(tt[:], T_d[fo])
                T_tiles.append(tt)

            # ---- GEMM: m_all [p=(f8,k), (i, f_o)], m_sh [p, (j, f_o)] ----
            m_all = mmp.tile([128, B, FO], BF16)
            m_sh = mmp.tile([128, JB, FO], BF16)
            for fo in range(FO):
                pm = gps.tile([128, B + JB], FP32, tag="gemm_full")
                for c in range(KC):
                    nc.tensor.matmul(
                        pm[:],
                        T_tiles[fo][:, c, :],
                        xT_sb[:, c, :],
                        start=(c == 0),
                        stop=(c == KC - 1),
                    )
                nc.scalar.copy(m_all[:, :, fo], pm[:, :B])
                nc.vector.tensor_copy(m_sh[:, :, fo], pm[:, B:])

            # ---- main pairwise loop ----
            acc = aps.tile([F8, JB * FO], FP32)  # sum over i of exp(-dist)
            sub_idx = 0
            for blk in range(NBLK):
                pd = dps.tile([128, JB * FO], FP32, tag="dist")
                for h in range(16 // SUBI):
                    i0 = blk * 16 + h * SUBI
                    diff = wp.tile([128, SUBI, JB, FO], BF16, tag="diff")
                    nc.vector.tensor_sub(
                        diff[:],
                        m_sh[:, None, :, :].broadcast_to([128, SUBI, JB, FO]),
                        m_all[:, i0:i0 + SUBI, None, :].broadcast_to(
                            [128, SUBI, JB, FO]
                        ),
                    )
                    ad = wp.tile([128, SUBI, JB, FO], BF16, tag="absd")
                    na = ACT_ABS_N
                    if na > 0:
                        nc.scalar.activation(
                            ad[:, :na], diff[:, :na],
                            mybir.ActivationFunctionType.Abs, bias=zero_b[:],
                        )
                    if na < SUBI:
                        # |x| on DVE at 4x mode: strip the bf16 sign bit
                        nc.vector.tensor_scalar(
                            ad[:, na:].bitcast(mybir.dt.uint16),
                            diff[:, na:].bitcast(mybir.dt.uint16),
                            0x7FFF, None, op0=mybir.AluOpType.bitwise_and,
                        )
                    sub_idx += 1
                    for s in range(SUBI):
                        isub = h * SUBI + s
                        g, q = isub // 8, isub % 8
                        nc.tensor.matmul(
                            pd[g * 64:(g + 1) * 64, :],
                            ones_k[:, q, :],
                            ad[:, s, :, :],
                            start=(q == 0),
                            stop=(q == 7),
                        )
                et = ep.tile([128, JB * FO], BF16, tag="expt")
                nc.scalar.activation(
                    et[:], pd[:],
                    mybir.ActivationFunctionType.Exp, bias=zero_b[:], scale=-1.0,
                )
                nc.tensor.matmul(
                    acc[:],
                    ones_a[:],
                    et[:],
                    start=(blk == 0),
                    stop=(blk == NBLK - 1),
                    skip_group_check=True,
                )

            # ---- tail: subtract 1, store ----
            fin = mmp.tile([F8, JB * FO], FP32)
            nc.vector.tensor_scalar_add(fin[:], acc[:], -1.0)
            nc.sync.dma_start(out_d[:], fin[:])

    nc.finalize()
    return nc


def make_in_maps(x: np.ndarray, T: np.ndarray):
    # xT_h[p, c, i] = x[i, c*128+p]
    xT_h = np.ascontiguousarray(
        x.T.astype(NPBF16).reshape(KC, 128, B).transpose(1, 0, 2)
    )
    T_b = np.ascontiguousarray(T).astype(NPBF16)           # [512, 2048]

    p = np.arange(128)[:, None]
    r = np.arange(F8)[None, :]
    ones_a = np.ascontiguousarray((p % 8 == r).astype(NPBF16))    # [128,8]
    # ones_k[p, q8, q] = 1 iff q == q8*8 + p//16  (q in 0..63)
    q = np.arange(64)[None, None, :]
    s = np.arange(8)[None, :, None]
    ones_k = (q == s * 8 + p[:, :, None] // 16).astype(NPBF16)
    ones_k = np.ascontiguousarray(ones_k.reshape(128, 8 * 64))

    # T_w host-permuted to [fo, p, c, n]: T_perm[fo, p, c, n] = T[c*128+p, fo*128+n]
    T_perm = np.ascontiguousarray(
        T_b.reshape(KC, 128, FO, 128).transpose(2, 1, 0, 3)
    )

    in_maps = []
    for c in range(N_CORES):
        xTc = np.ascontiguousarray(np.concatenate(
            [xT_h, xT_h[:, :, c * JB:(c + 1) * JB]], axis=2
        ))
        in_maps.append({
            "xT": xTc,
            "T_w": T_perm,
            "ones_k": ones_k,
            "ones_acc": ones_a,
        })
    return in_maps


def assemble(x: np.ndarray, pair_parts) -> np.ndarray:
    """pair_parts: list of [8, JB*FO] fp32 per core -> full [B, IN_F+OUT_F]."""
    out = np.empty((B, IN_F + OUT_F), np.float32)
    out[:, :IN_F] = x
    for c, fp in enumerate(pair_parts):
        # fp[f8, j*FO + fo] -> out[c*JB + j, IN_F + fo*8 + f8]
        blk = fp.reshape(F8, JB, FO).transpose(1, 2, 0).reshape(JB, OUT_F)
        out[c * JB:(c + 1) * JB, IN_F:] = blk
    return out


_NC_CACHE = None


def kernel(x: np.ndarray, T: np.ndarray) -> np.ndarray:
    global _NC_CACHE
    from concourse import bass_utils

    if _NC_CACHE is None:
        _NC_CACHE = build_nc()
    nc = _NC_CACHE
    in_maps = make_in_maps(np.asarray(x, np.float32), np.asarray(T, np.float32))
    res = bass_utils.run_bass_kernel_spmd(nc, in_maps, core_ids=list(range(N_CORES)))
    parts = [r["out_pair"].astype(np.float32) for r in res.results]
    return assemble(np.asarray(x, np.float32), parts)



# revision 3
# speedup vs baseline: 1.0326x; 1.0326x over previous
"""MiniBatchDiscrimination Trainium2 kernel (8-core SPMD).

Reference computation:
    m = (x @ T).reshape(B, OUT_F, NUM_K)            # B=256, OUT_F=128, NUM_K=16
    dists = |m[None,:,:,:] - m[:,None,:,:]|         # [B, B, OUT_F, NUM_K]
    out = sum_i exp(-sum_k dists) - 1               # [B, OUT_F]
    return concat([x, out], axis=-1)                # [B, 640]

Strategy (per core, identical SPMD program, per-core data):
  * Each core owns JB=32 output rows (j); full m replicated (cheap GEMM).
  * m stored as [p=(f8,k), i, fo] with f = fo*8 + f8, p = f8*16 + k.
  * Max-trick: |a-b| = 2*max(a,b) - a - b, so with M_ij = sum_k max and
    s_i = sum_k m[i,f,k]:
        exp(-d_ij) = exp(-2*M_ij) * exp(s_i) * exp(s_j)
    This removes the abs pass entirely: DVE does ONE tensor_max per
    16-i block (2x mode), TensorE does the k-sum (same ones_k matmuls as
    the distance path), ACT does exp(-2*M), one small DVE multiply folds
    exp(s_i), and exp(s_j) is applied once at the end.
  * s_i is computed with the same ones_k matmul patterns on m itself;
    the diagonal stays near-exact because max(x,x)=x and both matmuls
    accumulate the identical 16 bf16 values in the same order.
  * sum over i: ones_acc matmul accumulating across all 16 blocks.
  * Host unshards: reshape to [32,128] per core, concat with x.
"""

import os
import numpy as np

import concourse.bass as bass
import concourse.tile as tile
from concourse import bacc, mybir

BF16 = mybir.dt.bfloat16
FP32 = mybir.dt.float32
NPBF16 = np.dtype(mybir.dt.np(BF16))

B = 256
IN_F = 512
OUT_F = 128
NUM_K = 16
N_CORES = 8
JB = B // N_CORES          # 32 j-rows owned per core
F8 = 8                     # f8 = f % 8   (partition group)
FO = OUT_F // F8           # 16 f_o values (free dim)
KC = IN_F // 128           # 4 contraction chunks for the GEMM
NBLK = B // 16             # 16 i-blocks of 16
# trailing i's per block whose max runs on GpSimd instead of DVE
GPSIMD_S = int(os.environ.get("GPSIMD_S", "0"))

EXP = mybir.ActivationFunctionType.Exp


def build_nc():
    nc = bacc.Bacc(name="minibatch_discrim")

    # host-prearranged [p, c, i]; columns B..B+JB repeat this core's own
    # j-columns so the diagonal of the pair matrix is exact.
    xT_d = nc.dram_tensor("xT", [128, KC, B + JB], BF16, kind="ExternalInput")
    # T_w[p, fo, c, n] = T[c*128+p, fo*128+n]; 16KB/partition, 2 DMAs.
    T_d = nc.dram_tensor("T_w", [128, FO * KC * 128], BF16, kind="ExternalInput")
    # [512 ones_k | 8 ones_acc | 8 ones_s8] packed into one DMA.
    ones_d = nc.dram_tensor("ones_pack", [128, 528], BF16, kind="ExternalInput")
    out_d = nc.dram_tensor("out_pair", [F8, JB, FO], FP32, kind="ExternalOutput")

    with tile.TileContext(nc) as tc:
        with (
            tc.tile_pool(name="const", bufs=1) as constp,
            tc.tile_pool(name="mm", bufs=1) as mmp,
            tc.tile_pool(name="gpsum", bufs=2, space=bass.MemorySpace.PSUM) as gps,
            tc.tile_pool(name="sp1", bufs=1, space=bass.MemorySpace.PSUM) as sp1,
            tc.tile_pool(name="sp2", bufs=1, space=bass.MemorySpace.PSUM) as sp2,
            tc.tile_pool(name="dpsum", bufs=3, space=bass.MemorySpace.PSUM) as dps,
            tc.tile_pool(name="apsum", bufs=1, space=bass.MemorySpace.PSUM) as aps,
            tc.tile_pool(name="work", bufs=2) as wp,
            tc.tile_pool(name="expp", bufs=3) as ep,
        ):
            # ---- inputs to SBUF: T first so the GEMM can start ASAP ----
            T_sb = constp.tile([128, FO, KC, 128], BF16)
            Tv = T_d.rearrange("p (fo c n) -> p fo c n", fo=FO, c=KC)
            h = FO // 2
            nc.sync.dma_start(T_sb[:, :h], Tv[:, :h])
            nc.sync.dma_start(T_sb[:, h:], Tv[:, h:])
            xT_sb = constp.tile([128, KC, B + JB], BF16)
            nc.sync.dma_start(xT_sb[:], xT_d[:])
            ones_sb = constp.tile([128, 528], BF16)
            nc.sync.dma_start(ones_sb[:], ones_d[:])

            zero_b = constp.tile([128, 1], FP32)
            nc.gpsimd.memset(zero_b[:], 0.0)
            # warm the ACT exp table while DMAs run
            warm = constp.tile([128, 1], FP32)
            nc.scalar.activation(warm[:], zero_b[:], EXP, bias=zero_b[:])

            # ---- GEMM: m_full [p=(f8,k), i(288), fo] (cols B.. = own j) ----
            m_full = mmp.tile([128, B + JB, FO], BF16)
            for fo in range(FO):
                pm = gps.tile([128, B + JB], FP32, tag="gemm")
                for c in range(KC):
                    nc.tensor.matmul(
                        pm[:],
                        T_sb[:, fo, c, :],
                        xT_sb[:, c, :],
                        start=(c == 0),
                        stop=(c == KC - 1),
                    )
                # strided-dst copy on DVE (1x mode, ~450ns) -- keeps ACT free
                nc.vector.tensor_copy(m_full[:, :, fo], pm[:])

            # ---- s_i = sum_k m: same ones_k patterns as the dist matmuls ----
            # s_ps[p=(g,q,f8), blk, fo] = s_i for i = blk*16 + g*8 + q
            m_gq = m_full[:, :B, :].rearrange("p (blk gq) fo -> p gq blk fo", gq=16)
            s_ps = sp1.tile([128, NBLK, FO], FP32)
            for g in range(2):
                for q in range(8):
                    nc.tensor.matmul(
                        s_ps[g * 64:(g + 1) * 64],
                        ones_sb[:, q * 64:(q + 1) * 64],
                        m_gq[:, g * 8 + q],
                        start=(q == 0),
                        stop=(q == 7),
                    )
            c_all = mmp.tile([128, NBLK, FO], BF16)
            nc.scalar.activation(c_all[:], s_ps[:], EXP, bias=zero_b[:], scale=1.0)

            # s_j for this core's own 32 columns -> c_sh [8, j, fo] fp32
            ssh_ps = sp2.tile([F8, JB, FO], FP32)
            nc.tensor.matmul(
                ssh_ps[:], ones_sb[:, 520:528], m_full[:, B:, :],
                start=True, stop=True,
            )
            c_sh = mmp.tile([F8, JB, FO], FP32)
            nc.scalar.activation(
                c_sh[:], ssh_ps[:], EXP, bias=zero_b[:F8], scale=1.0
            )

            # ---- main pairwise loop ----
            acc = aps.tile([F8, JB, FO], FP32)  # sum over i of exp-terms
            nd = 16 - GPSIMD_S
            for blk in range(NBLK):
                i0 = blk * 16
                mx = wp.tile([128, 16, JB, FO], BF16, tag="mx")
                nc.vector.tensor_max(
                    mx[:, :nd],
                    m_full[:, None, B:, :].broadcast_to([128, nd, JB, FO]),
                    m_full[:, i0:i0 + nd, None, :].broadcast_to(
                        [128, nd, JB, FO]
                    ),
                )
                if GPSIMD_S:
                    nc.gpsimd.tensor_max(
                        mx[:, nd:],
                        m_full[:, None, B:, :].broadcast_to(
                            [128, GPSIMD_S, JB, FO]
                        ),
                        m_full[:, i0 + nd:i0 + 16, None, :].broadcast_to(
                            [128, GPSIMD_S, JB, FO]
                        ),
                    )
                pd = dps.tile([128, JB, FO], FP32, tag="dist")
                for s in range(16):
                    g, q = s // 8, s % 8
                    nc.tensor.matmul(
                        pd[g * 64:(g + 1) * 64],
                        ones_sb[:, q * 64:(q + 1) * 64],
                        mx[:, s],
                        start=(q == 0),
                        stop=(q == 7),
                    )
                et = ep.tile([128, JB, FO], BF16, tag="et")
                nc.scalar.activation(et[:], pd[:], EXP, bias=zero_b[:], scale=-2.0)
                et2 = ep.tile([128, JB, FO], BF16, tag="et2")
                nc.vector.tensor_mul(
                    et2[:],
                    et[:],
                    c_all[:, blk, None, :].broadcast_to([128, JB, FO]),
                )
                nc.tensor.matmul(
                    acc[:],
                    ones_sb[:, 512:520],
                    et2[:],
                    start=(blk == 0),
                    stop=(blk == NBLK - 1),
                    skip_group_check=True,
                )

            # ---- tail: * exp(s_j), subtract 1, store ----
            fin = mmp.tile([F8, JB, FO], FP32)
            nc.vector.tensor_mul(fin[:], acc[:], c_sh[:])
            fin2 = mmp.tile([F8, JB, FO], FP32)
            nc.vector.tensor_scalar_add(fin2[:], fin[:], -1.0)
            nc.sync.dma_start(out_d[:], fin2[:])

    nc.finalize()
    return nc


def make_in_maps(x: np.ndarray, T: np.ndarray):
    # xT_h[p, c, i] = x[i, c*128+p]
    xT_h = np.ascontiguousarray(
        x.T.astype(NPBF16).reshape(KC, 128, B).transpose(1, 0, 2)
    )
    T_b = np.ascontiguousarray(T).astype(NPBF16)           # [512, 2048]
    # T_w[p, fo, c, n] = T[c*128+p, fo*128+n]
    T_perm = np.ascontiguousarray(
        T_b.reshape(KC, 128, FO, 128).transpose(1, 2, 0, 3)
    ).reshape(128, FO * KC * 128)

    p = np.arange(128)[:, None]
    r = np.arange(F8)[None, :]
    ones_a = (p % 8 == r).astype(NPBF16)                   # [128, 8]
    ones_s8 = (p // 16 == r).astype(NPBF16)                # [128, 8]
    # ones_k[p, q8, q] = 1 iff q == q8*8 + p//16  (q in 0..63)
    q = np.arange(64)[None, None, :]
    s = np.arange(8)[None, :, None]
    ones_k = (q == s * 8 + p[:, :, None] // 16).astype(NPBF16).reshape(128, 512)
    ones_pack = np.ascontiguousarray(
        np.concatenate([ones_k, ones_a, ones_s8], axis=1)
    )

    in_maps = []
    for c in range(N_CORES):
        xTc = np.ascontiguousarray(np.concatenate(
            [xT_h, xT_h[:, :, c * JB:(c + 1) * JB]], axis=2
        ))
        in_maps.append({
            "xT": xTc,
            "T_w": T_perm,
            "ones_pack": ones_pack,
        })
    return in_maps


def assemble(x: np.ndarray, pair_parts) -> np.ndarray:
    """pair_parts: list of [8, JB, FO] fp32 per core -> full [B, IN_F+OUT_F]."""
    out = np.empty((B, IN_F + OUT_F), np.float32)
    out[:, :IN_F] = x
    for c, fp in enumerate(pair_parts):
        # fp[f8, j, fo] -> out[c*JB + j, IN_F + fo*8 + f8]
        blk = fp.reshape(F8, JB, FO).transpose(1, 2, 0).reshape(JB, OUT_F)
        out[c * JB:(c + 1) * JB, IN_F:] = blk
    return out


_NC_CACHE = None


def kernel(x: np.ndarray, T: np.ndarray) -> np.ndarray:
    global _NC_CACHE
    from concourse import bass_utils

    if _NC_CACHE is None:
        _NC_CACHE = build_nc()
    nc = _NC_CACHE
    in_maps = make_in_maps(np.asarray(x, np.float32), np.asarray(T, np.float32))
    res = bass_utils.run_bass_kernel_spmd(nc, in_maps, core_ids=list(range(N_CORES)))
    parts = [r["out_pair"].astype(np.float32) for r in res.results]
    return assemble(np.asarray(x, np.float32), parts)


# revision 9
# speedup vs baseline: 1.1105x; 1.0755x over previous
"""MiniBatchDiscrimination Trainium2 kernel (8-core SPMD).

Reference computation:
    m = (x @ T).reshape(B, OUT_F, NUM_K)            # B=256, OUT_F=128, NUM_K=16
    dists = |m[None,:,:,:] - m[:,None,:,:]|         # [B, B, OUT_F, NUM_K]
    out = sum_i exp(-sum_k dists) - 1               # [B, OUT_F]
    return concat([x, out], axis=-1)                # [B, 640]

Strategy (per core, identical SPMD program, per-core data):
  * Each core owns JB=32 output rows (j); full m replicated (cheap GEMM).
  * m stored as [p=(f8,k), i, fo] with f = fo*8 + f8, p = f8*16 + k.
  * Max-trick: |a-b| = 2*max(a,b) - a - b, so with M_ij = sum_k max and
    s_i = sum_k m[i,f,k]:
        exp(-d_ij) = exp(-2*M_ij) * exp(s_i) * exp(s_j)
    This removes the abs pass entirely: DVE does ONE tensor_max per
    16-i block (2x mode), TensorE does the k-sum (same ones_k matmuls as
    the distance path), ACT does exp(-2*M), one small DVE multiply folds
    exp(s_i), and exp(s_j) is applied once at the end.
  * s_i is computed with the same ones_k matmul patterns on m itself;
    the diagonal stays near-exact because max(x,x)=x and both matmuls
    accumulate the identical 16 bf16 values in the same order.
  * sum over i: ones_acc matmul accumulating across all 16 blocks.
  * Host unshards: reshape to [32,128] per core, concat with x.
"""

import os
import numpy as np

import concourse.bass as bass
import concourse.tile as tile
from concourse import bacc, mybir

BF16 = mybir.dt.bfloat16
FP32 = mybir.dt.float32
NPBF16 = np.dtype(mybir.dt.np(BF16))

B = 256
IN_F = 512
OUT_F = 128
NUM_K = 16
N_CORES = 8
JB = B // N_CORES          # 32 j-rows owned per core
F8 = 8                     # f8 = f % 8   (partition group)
FO = OUT_F // F8           # 16 f_o values (free dim)
KC = IN_F // 128           # 4 contraction chunks for the GEMM
NBLK = B // 16             # 16 i-blocks of 16
# trailing i's per block whose max runs on GpSimd instead of DVE
GPSIMD_S = int(os.environ.get("GPSIMD_S", "3"))
# dummy matmuls issued during the input DMA wait to pull the PE out of its
# cold 1.2GHz HAM state before the real GEMM begins (~5us of filler)
WARM_MM = int(os.environ.get("WARM_MM", "20"))

EXP = mybir.ActivationFunctionType.Exp


def build_nc():
    nc = bacc.Bacc(name="minibatch_discrim")

    # host-prearranged [p, c, i]; columns B..B+JB repeat this core's own
    # j-columns so the diagonal of the pair matrix is exact.
    xT_d = nc.dram_tensor("xT", [128, KC, B + JB], BF16, kind="ExternalInput")
    # T_w[p, fo, c, n] = T[c*128+p, fo*128+n]; 16KB/partition, 2 DMAs.
    T_d = nc.dram_tensor("T_w", [128, FO * KC * 128], BF16, kind="ExternalInput")
    # [512 ones_k | 8 ones_acc | 8 ones_s8] packed into one DMA.
    ones_d = nc.dram_tensor("ones_pack", [128, 528], BF16, kind="ExternalInput")
    out_d = nc.dram_tensor("out_pair", [F8, JB, FO], FP32, kind="ExternalOutput")

    with tile.TileContext(nc) as tc:
        with (
            tc.tile_pool(name="const", bufs=1) as constp,
            tc.tile_pool(name="mm", bufs=1) as mmp,
            tc.tile_pool(name="gpsum", bufs=2, space=bass.MemorySpace.PSUM) as gps,
            tc.tile_pool(name="sp1", bufs=1, space=bass.MemorySpace.PSUM) as sp1,
            tc.tile_pool(name="sp2", bufs=1, space=bass.MemorySpace.PSUM) as sp2,
            tc.tile_pool(name="dpsum", bufs=3, space=bass.MemorySpace.PSUM) as dps,
            tc.tile_pool(name="apsum", bufs=1, space=bass.MemorySpace.PSUM) as aps,
            tc.tile_pool(name="work", bufs=2) as wp,
            tc.tile_pool(name="expp", bufs=3) as ep,
        ):
            # ---- inputs to SBUF: xT first (moving operand), then T halves.
            # Flat 2D slices keep the DMA element size large (8-16KB lines).
            xT_sb = constp.tile([128, KC, B + JB], BF16)
            nc.sync.dma_start(xT_sb[:], xT_d[:])
            T_sb = constp.tile([128, FO * KC * 128], BF16)
            hcol = (FO // 2) * KC * 128
            nc.sync.dma_start(T_sb[:, :hcol], T_d[:, :hcol])
            nc.sync.dma_start(T_sb[:, hcol:], T_d[:, hcol:])
            ones_sb = constp.tile([128, 528], BF16)
            nc.sync.dma_start(ones_sb[:], ones_d[:])

            zero_b = constp.tile([128, 1], FP32)
            nc.gpsimd.memset(zero_b[:], 0.0)
            # warm the ACT exp table while DMAs run
            warm = constp.tile([128, 1], FP32)
            nc.scalar.activation(warm[:], zero_b[:], EXP, bias=zero_b[:])

            # PE warm-up during the DMA wait: the HAM clock gate defaults to
            # 1.2GHz and needs ~3.4us of sustained matmul activity to release.
            if WARM_MM:
                wz = constp.tile([128, 512], BF16)
                nc.gpsimd.memset(wz[:], 0.0)
                wpd = gps.tile([128, B + JB], FP32, tag="gemm")
                for w in range(WARM_MM):
                    nc.tensor.matmul(
                        wpd[:], wz[:, :128], wz[:, :B + JB],
                        start=(w == 0), stop=(w == WARM_MM - 1),
                    )

            # ---- GEMM: m_full [p=(f8,k), i(288), fo] (cols B.. = own j) ----
            m_full = mmp.tile([128, B + JB, FO], BF16)
            for fo in range(FO):
                pm = gps.tile([128, B + JB], FP32, tag="gemm")
                for c in range(KC):
                    base = (fo * KC + c) * 128
                    nc.tensor.matmul(
                        pm[:],
                        T_sb[:, base:base + 128],
                        xT_sb[:, c, :],
                        start=(c == 0),
                        stop=(c == KC - 1),
                    )
                # strided-dst copies are slow (~1.5us) on either engine;
                # alternate ACT/DVE so the two streams overlap.
                if fo % 2 == 0:
                    nc.vector.tensor_copy(m_full[:, :, fo], pm[:])
                else:
                    nc.scalar.copy(m_full[:, :, fo], pm[:])

            # ---- s_i = sum_k m: same ones_k patterns as the dist matmuls ----
            # s_ps[p=(g,q,f8), blk, fo] = s_i for i = blk*16 + g*8 + q
            m_gq = m_full[:, :B, :].rearrange("p (blk gq) fo -> p gq blk fo", gq=16)
            s_ps = sp1.tile([128, NBLK, FO], FP32)
            for g in range(2):
                for q in range(8):
                    nc.tensor.matmul(
                        s_ps[g * 64:(g + 1) * 64],
                        ones_sb[:, q * 64:(q + 1) * 64],
                        m_gq[:, g * 8 + q],
                        start=(q == 0),
                        stop=(q == 7),
                    )
            c_all = mmp.tile([128, NBLK, FO], BF16)
            nc.scalar.activation(c_all[:], s_ps[:], EXP, bias=zero_b[:], scale=1.0)

            # s_j for this core's own 32 columns -> c_sh [8, j, fo] fp32
            ssh_ps = sp2.tile([F8, JB, FO], FP32)
            nc.tensor.matmul(
                ssh_ps[:], ones_sb[:, 520:528], m_full[:, B:, :],
                start=True, stop=True,
            )
            c_sh = mmp.tile([F8, JB, FO], FP32)
            nc.scalar.activation(
                c_sh[:], ssh_ps[:], EXP, bias=zero_b[:F8], scale=1.0
            )

            # ---- main pairwise loop ----
            acc = aps.tile([F8, JB, FO], FP32)  # sum over i of exp-terms
            nd = 16 - GPSIMD_S
            for blk in range(NBLK):
                i0 = blk * 16
                mx = wp.tile([128, 16, JB, FO], BF16, tag="mx")
                nc.vector.tensor_max(
                    mx[:, :nd],
                    m_full[:, None, B:, :].broadcast_to([128, nd, JB, FO]),
                    m_full[:, i0:i0 + nd, None, :].broadcast_to(
                        [128, nd, JB, FO]
                    ),
                )
                if GPSIMD_S:
                    nc.gpsimd.tensor_max(
                        mx[:, nd:],
                        m_full[:, None, B:, :].broadcast_to(
                            [128, GPSIMD_S, JB, FO]
                        ),
                        m_full[:, i0 + nd:i0 + 16, None, :].broadcast_to(
                            [128, GPSIMD_S, JB, FO]
                        ),
                    )
                pd = dps.tile([128, JB, FO], FP32, tag="dist")
                for s in range(16):
                    g, q = s // 8, s % 8
                    nc.tensor.matmul(
                        pd[g * 64:(g + 1) * 64],
                        ones_sb[:, q * 64:(q + 1) * 64],
                        mx[:, s],
                        start=(q == 0),
                        stop=(q == 7),
                    )
                et = ep.tile([128, JB, FO], BF16, tag="et")
                nc.scalar.activation(et[:], pd[:], EXP, bias=zero_b[:], scale=-2.0)
                et2 = ep.tile([128, JB, FO], BF16, tag="et2")
                nc.vector.tensor_mul(
                    et2[:],
                    et[:],
                    c_all[:, blk, None, :].broadcast_to([128, JB, FO]),
                )
                nc.tensor.matmul(
                    acc[:],
                    ones_sb[:, 512:520],
                    et2[:],
                    start=(blk == 0),
                    stop=(blk == NBLK - 1),
                    skip_group_check=True,
                )

            # ---- tail: * exp(s_j), subtract 1, store ----
            fin = mmp.tile([F8, JB, FO], FP32)
            nc.vector.tensor_mul(fin[:], acc[:], c_sh[:])
            fin2 = mmp.tile([F8, JB, FO], FP32)
            nc.vector.tensor_scalar_add(fin2[:], fin[:], -1.0)
            nc.sync.dma_start(out_d[:], fin2[:])

    nc.finalize()
    return nc


def make_in_maps(x: np.ndarray, T: np.ndarray):
    # xT_h[p, c, i] = x[i, c*128+p]
    xT_h = np.ascontiguousarray(
        x.T.astype(NPBF16).reshape(KC, 128, B).transpose(1, 0, 2)
    )
    T_b = np.ascontiguousarray(T).astype(NPBF16)           # [512, 2048]
    # T_w[p, fo, c, n] = T[c*128+p, fo*128+n]
    T_perm = np.ascontiguousarray(
        T_b.reshape(KC, 128, FO, 128).transpose(1, 2, 0, 3)
    ).reshape(128, FO * KC * 128)

    p = np.arange(128)[:, None]
    r = np.arange(F8)[None, :]
    ones_a = (p % 8 == r).astype(NPBF16)                   # [128, 8]
    ones_s8 = (p // 16 == r).astype(NPBF16)                # [128, 8]
    # ones_k[p, q8, q] = 1 iff q == q8*8 + p//16  (q in 0..63)
    q = np.arange(64)[None, None, :]
    s = np.arange(8)[None, :, None]
    ones_k = (q == s * 8 + p[:, :, None] // 16).astype(NPBF16).reshape(128, 512)
    ones_pack = np.ascontiguousarray(
        np.concatenate([ones_k, ones_a, ones_s8], axis=1)
    )

    in_maps = []
    for c in range(N_CORES):
        xTc = np.ascontiguousarray(np.concatenate(
            [xT_h, xT_h[:, :, c * JB:(c + 1) * JB]], axis=2
        ))
        in_maps.append({
            "xT": xTc,
            "T_w": T_perm,
            "ones_pack": ones_pack,
        })
    return in_maps


def assemble(x: np.ndarray, pair_parts) -> np.ndarray:
    """pair_parts: list of [8, JB, FO] fp32 per core -> full [B, IN_F+OUT_F]."""
    out = np.empty((B, IN_F + OUT_F), np.float32)
    out[:, :IN_F] = x
    for c, fp in enumerate(pair_parts):
        # fp[f8, j, fo] -> out[c*JB + j, IN_F + fo*8 + f8]
        blk = fp.reshape(F8, JB, FO).transpose(1, 2, 0).reshape(JB, OUT_F)
        out[c * JB:(c + 1) * JB, IN_F:] = blk
    return out


_NC_CACHE = None


def kernel(x: np.ndarray, T: np.ndarray) -> np.ndarray:
    global _NC_CACHE
    from concourse import bass_utils

    if _NC_CACHE is None:
        _NC_CACHE = build_nc()
    nc = _NC_CACHE
    in_maps = make_in_maps(np.asarray(x, np.float32), np.asarray(T, np.float32))
    res = bass_utils.run_bass_kernel_spmd(nc, in_maps, core_ids=list(range(N_CORES)))
    parts = [r["out_pair"].astype(np.float32) for r in res.results]
    return assemble(np.asarray(x, np.float32), parts)


# revision 16
# speedup vs baseline: 1.2045x; 1.0846x over previous
"""MiniBatchDiscrimination Trainium2 kernel (8-core SPMD).

Reference computation:
    m = (x @ T).reshape(B, OUT_F, NUM_K)            # B=256, OUT_F=128, NUM_K=16
    dists = |m[None,:,:,:] - m[:,None,:,:]|         # [B, B, OUT_F, NUM_K]
    out = sum_i exp(-sum_k dists) - 1               # [B, OUT_F]
    return concat([x, out], axis=-1)                # [B, 640]

Strategy (per core, identical SPMD program, per-core data):
  * Each core owns JB=32 output rows (j); full m replicated (cheap GEMM).
  * m stored as [p=(f8,k), i, fo] with f = fo*8 + f8, p = f8*16 + k.
  * Max-trick: |a-b| = 2*max(a,b) - a - b, so with M_ij = sum_k max and
    s_i = sum_k m[i,f,k]:
        exp(-d_ij) = exp(-2*M_ij) * exp(s_i) * exp(s_j)
    This removes the abs pass entirely: DVE does ONE tensor_max per
    16-i block (2x mode), TensorE does the k-sum (same ones_k matmuls as
    the distance path), ACT does exp(-2*M), one small DVE multiply folds
    exp(s_i), and exp(s_j) is applied once at the end.
  * s_i is computed with the same ones_k matmul patterns on m itself;
    the diagonal stays near-exact because max(x,x)=x and both matmuls
    accumulate the identical 16 bf16 values in the same order.
  * sum over i: ones_acc matmul accumulating across all 16 blocks.
  * Host unshards: reshape to [32,128] per core, concat with x.
"""

import os
import numpy as np

import concourse.bass as bass
import concourse.tile as tile
from concourse import bacc, mybir

BF16 = mybir.dt.bfloat16
FP32 = mybir.dt.float32
NPBF16 = np.dtype(mybir.dt.np(BF16))

B = 256
IN_F = 512
OUT_F = 128
NUM_K = 16
N_CORES = 8
JB = B // N_CORES          # 32 j-rows owned per core
F8 = 8                     # f8 = f % 8   (partition group)
FO = OUT_F // F8           # 16 f_o values (free dim)
KC = IN_F // 128           # 4 contraction chunks for the GEMM
NBLK = B // 16             # 16 i-blocks of 16
# trailing i's per block whose max runs on GpSimd instead of DVE
GPSIMD_S = int(os.environ.get("GPSIMD_S", "0"))
# dummy matmuls issued during the input DMA wait to pull the PE out of its
# cold 1.2GHz HAM state before the real GEMM begins (~5us of filler)
WARM_MM = int(os.environ.get("WARM_MM", "20"))

EXP = mybir.ActivationFunctionType.Exp


def build_nc():
    nc = bacc.Bacc(name="minibatch_discrim")

    # host-prearranged [p, c, i]; columns B..B+JB repeat this core's own
    # j-columns so the diagonal of the pair matrix is exact.
    xT_d = nc.dram_tensor("xT", [128, KC, B + JB], BF16, kind="ExternalInput")
    # T_w[p, fo, c, n] = T[c*128+p, fo*128+n]; 16KB/partition, 2 DMAs.
    T_d = nc.dram_tensor("T_w", [128, FO * KC * 128], BF16, kind="ExternalInput")
    # [512 ones_k | 8 ones_acc | 8 ones_s8] packed into one DMA.
    ones_d = nc.dram_tensor("ones_pack", [128, 528], BF16, kind="ExternalInput")
    # fp32 identity for the s_i-fold matmul (fp32 keeps the diagonal exact)
    ident_d = nc.dram_tensor("ident", [128, 128], FP32, kind="ExternalInput")
    out_d = nc.dram_tensor("out_pair", [F8, JB, FO], FP32, kind="ExternalOutput")

    with tile.TileContext(nc) as tc:
        with (
            tc.tile_pool(name="const", bufs=1) as constp,
            tc.tile_pool(name="mm", bufs=1) as mmp,
            tc.tile_pool(name="gpsum", bufs=3, space=bass.MemorySpace.PSUM) as gps,
            tc.tile_pool(name="sp1", bufs=1, space=bass.MemorySpace.PSUM) as sp1,
            tc.tile_pool(name="sp2", bufs=1, space=bass.MemorySpace.PSUM) as sp2,
            tc.tile_pool(name="dpsum", bufs=2, space=bass.MemorySpace.PSUM) as dps,
            tc.tile_pool(name="apsum", bufs=1, space=bass.MemorySpace.PSUM) as aps,
            tc.tile_pool(name="work", bufs=2) as wp,
            tc.tile_pool(name="expp", bufs=3) as ep,
        ):
            # ---- inputs to SBUF: xT first (moving operand), then T halves.
            # Flat 2D slices keep the DMA element size large (8-16KB lines).
            xT_sb = constp.tile([128, KC, B + JB], BF16)
            nc.sync.dma_start(xT_sb[:], xT_d[:])
            T_sb = constp.tile([128, FO * KC * 128], BF16)
            hcol = (FO // 2) * KC * 128
            nc.sync.dma_start(T_sb[:, :hcol], T_d[:, :hcol])
            nc.sync.dma_start(T_sb[:, hcol:], T_d[:, hcol:])
            ones_sb = constp.tile([128, 528], BF16)
            nc.sync.dma_start(ones_sb[:], ones_d[:])
            ident_sb = constp.tile([128, 128], FP32)
            nc.sync.dma_start(ident_sb[:], ident_d[:])

            zero_b = constp.tile([128, 1], FP32)
            nc.gpsimd.memset(zero_b[:], 0.0)
            # warm the ACT exp table while DMAs run
            warm = constp.tile([128, 1], FP32)
            nc.scalar.activation(warm[:], zero_b[:], EXP, bias=zero_b[:])

            # PE warm-up during the DMA wait: the HAM clock gate defaults to
            # 1.2GHz and needs ~3.4us of sustained matmul activity to release.
            if WARM_MM:
                wz = constp.tile([128, 512], BF16)
                nc.gpsimd.memset(wz[:], 0.0)
                wpd = gps.tile([128, B + JB], FP32, tag="gemm")
                for w in range(WARM_MM):
                    nc.tensor.matmul(
                        wpd[:], wz[:, :128], wz[:, :B + JB],
                        start=(w == 0), stop=(w == WARM_MM - 1),
                    )

            # ---- GEMM: m_full [p=(f8,k), i(288), fo] (cols B.. = own j) ----
            m_full = mmp.tile([128, B + JB, FO], BF16)
            for fo in range(FO):
                pm = gps.tile([128, B + JB], FP32, tag="gemm")
                for c in range(KC):
                    base = (fo * KC + c) * 128
                    nc.tensor.matmul(
                        pm[:],
                        T_sb[:, base:base + 128],
                        xT_sb[:, c, :],
                        start=(c == 0),
                        stop=(c == KC - 1),
                    )
                # strided-dst copies are slow (~1.5us) on either engine;
                # alternate ACT/DVE so the two streams overlap.
                if fo % 2 == 0:
                    nc.vector.tensor_copy(m_full[:, :, fo], pm[:])
                else:
                    nc.scalar.copy(m_full[:, :, fo], pm[:])

            # ---- s_i = sum_k m: same ones_k patterns as the dist matmuls ----
            # s_ps[p=(g,q,f8), blk, fo] = s_i for i = blk*16 + g*8 + q
            m_gq = m_full[:, :B, :].rearrange("p (blk gq) fo -> p gq blk fo", gq=16)
            s_ps = sp1.tile([128, NBLK, FO], FP32)
            for g in range(2):
                for q in range(8):
                    nc.tensor.matmul(
                        s_ps[g * 64:(g + 1) * 64],
                        ones_sb[:, q * 64:(q + 1) * 64],
                        m_gq[:, g * 8 + q],
                        start=(q == 0),
                        stop=(q == 7),
                    )
            # -(s_i)/2 in fp32; added into each dist PSUM tile by an identity
            # matmul so the exp(scale=-2) directly yields exp(-2M + s_i).
            s_half = mmp.tile([128, NBLK, FO], FP32)
            nc.vector.tensor_scalar_mul(s_half[:], s_ps[:], -0.5)

            # s_j for this core's own 32 columns -> c_sh [8, j, fo] fp32
            ssh_ps = sp2.tile([F8, JB, FO], FP32)
            nc.tensor.matmul(
                ssh_ps[:], ones_sb[:, 520:528], m_full[:, B:, :],
                start=True, stop=True,
            )
            c_sh = mmp.tile([F8, JB, FO], FP32)
            nc.scalar.activation(
                c_sh[:], ssh_ps[:], EXP, bias=zero_b[:F8], scale=1.0
            )

            # ---- main pairwise loop ----
            acc = aps.tile([F8, JB, FO], FP32)  # sum over i of exp-terms
            nd = 16 - GPSIMD_S
            for blk in range(NBLK):
                i0 = blk * 16
                mx = wp.tile([128, 16, JB, FO], BF16, tag="mx")
                nc.vector.tensor_max(
                    mx[:, :nd],
                    m_full[:, None, B:, :].broadcast_to([128, nd, JB, FO]),
                    m_full[:, i0:i0 + nd, None, :].broadcast_to(
                        [128, nd, JB, FO]
                    ),
                )
                if GPSIMD_S:
                    nc.gpsimd.tensor_max(
                        mx[:, nd:],
                        m_full[:, None, B:, :].broadcast_to(
                            [128, GPSIMD_S, JB, FO]
                        ),
                        m_full[:, i0 + nd:i0 + 16, None, :].broadcast_to(
                            [128, GPSIMD_S, JB, FO]
                        ),
                    )
                pd = dps.tile([128, JB, FO], FP32, tag="dist")
                for s in range(16):
                    g, q = s // 8, s % 8
                    nc.tensor.matmul(
                        pd[g * 64:(g + 1) * 64],
                        ones_sb[:, q * 64:(q + 1) * 64],
                        mx[:, s],
                        start=(q == 0),
                        stop=False,
                    )
                # pd += -s_i/2 (identity stationary; j-broadcast moving)
                nc.tensor.matmul(
                    pd[:],
                    ident_sb[:],
                    s_half[:, blk, None, :].broadcast_to([128, JB, FO]),
                    start=False,
                    stop=True,
                    skip_group_check=True,
                )
                et = ep.tile([128, JB, FO], BF16, tag="et")
                nc.scalar.activation(et[:], pd[:], EXP, bias=zero_b[:], scale=-2.0)
                nc.tensor.matmul(
                    acc[:],
                    ones_sb[:, 512:520],
                    et[:],
                    start=(blk == 0),
                    stop=(blk == NBLK - 1),
                    skip_group_check=True,
                )

            # ---- tail: * exp(s_j), subtract 1, store ----
            fin = mmp.tile([F8, JB, FO], FP32)
            nc.vector.tensor_mul(fin[:], acc[:], c_sh[:])
            fin2 = mmp.tile([F8, JB, FO], FP32)
            nc.vector.tensor_scalar_add(fin2[:], fin[:], -1.0)
            nc.sync.dma_start(out_d[:], fin2[:])

    nc.finalize()
    return nc


def make_in_maps(x: np.ndarray, T: np.ndarray):
    # xT_h[p, c, i] = x[i, c*128+p]
    xT_h = np.ascontiguousarray(
        x.T.astype(NPBF16).reshape(KC, 128, B).transpose(1, 0, 2)
    )
    T_b = np.ascontiguousarray(T).astype(NPBF16)           # [512, 2048]
    # T_w[p, fo, c, n] = T[c*128+p, fo*128+n]
    T_perm = np.ascontiguousarray(
        T_b.reshape(KC, 128, FO, 128).transpose(1, 2, 0, 3)
    ).reshape(128, FO * KC * 128)

    p = np.arange(128)[:, None]
    r = np.arange(F8)[None, :]
    ones_a = (p % 8 == r).astype(NPBF16)                   # [128, 8]
    ones_s8 = (p // 16 == r).astype(NPBF16)                # [128, 8]
    # ones_k[p, q8, q] = 1 iff q == q8*8 + p//16  (q in 0..63)
    q = np.arange(64)[None, None, :]
    s = np.arange(8)[None, :, None]
    ones_k = (q == s * 8 + p[:, :, None] // 16).astype(NPBF16).reshape(128, 512)
    ones_pack = np.ascontiguousarray(
        np.concatenate([ones_k, ones_a, ones_s8], axis=1)
    )

    in_maps = []
    for c in range(N_CORES):
        xTc = np.ascontiguousarray(np.concatenate(
            [xT_h, xT_h[:, :, c * JB:(c + 1) * JB]], axis=2
        ))
        in_maps.append({
            "xT": xTc,
            "T_w": T_perm,
            "ones_pack": ones_pack,
            "ident": np.eye(128, dtype=np.float32),
        })
    return in_maps


def assemble(x: np.ndarray, pair_parts) -> np.ndarray:
    """pair_parts: list of [8, JB, FO] fp32 per core -> full [B, IN_F+OUT_F]."""
    out = np.empty((B, IN_F + OUT_F), np.float32)
    out[:, :IN_F] = x
    for c, fp in enumerate(pair_parts):
        # fp[f8, j, fo] -> out[c*JB + j, IN_F + fo*8 + f8]
        blk = fp.reshape(F8, JB, FO).transpose(1, 2, 0).reshape(JB, OUT_F)
        out[c * JB:(c + 1) * JB, IN_F:] = blk
    return out


_NC_CACHE = None


def kernel(x: np.ndarray, T: np.ndarray) -> np.ndarray:
    global _NC_CACHE
    from concourse import bass_utils

    if _NC_CACHE is None:
        _NC_CACHE = build_nc()
    nc = _NC_CACHE
    in_maps = make_in_maps(np.asarray(x, np.float32), np.asarray(T, np.float32))
    res = bass_utils.run_bass_kernel_spmd(nc, in_maps, core_ids=list(range(N_CORES)))
    parts = [r["out_pair"].astype(np.float32) for r in res.results]
    return assemble(np.asarray(x, np.float32), parts)
